# revision 13
# baseline (speedup 1.0000x reference)
"""Trainium2 Bass kernel for DeformConv2D (b=4, c=64, H=W=128, ks=3).

Sharding: 8 cores = (sample s = core//2) x (row-half = core%2). Each core
computes output rows [64*half, 64*half+64) of its sample.

v3 dataflow (per core), software-pipelined over 4 t-chunks of 16 rows:
  A. Load a 74-row bf16 halo slab of x (CHW) into SBUF; build XD2 in DRAM:
     [74*132 slots, 512B] where slot (k,c) = 2rows x 2cols x 64ch -- ONE
     gather descriptor fetches a full bilinear corner block per sample.
  B. Offset conv on PE (bf16, 9 taps, K=64); PE-transpose to [128w,16t,18].
  C. DVE coordinate pipeline -> masked fracs + linear slot idx; idx
     relayout to the gather's wrapped-16 layout via DRAM staging (the
     ph-interleave is folded into the staging-load DMA access pattern).
  D. 9 dma_gathers per chunk (2048 idxs, 512B elems, 6-deep buffer ring);
     combine = lerp-of-lerp: DVE batched corner deltas + fused row madds,
     ACT engine does the column-stage multiplies; PE transpose + final
     conv as 512-wide accumulating matmuls. Out-DMA rides the ACT queue
     so it never blocks the sync-queue idx staging.
  Chunk t+1's B/C phases are issued before chunk t's combine so the Pool
  engine's gather stream never starves.
"""
import sys
import types
import numpy as np
import ml_dtypes

sys.path.insert(0, "/opt/trn_rl_repo")

BF16 = ml_dtypes.bfloat16
NCORES = 8
NR = 74          # slab rows (local): row k <-> unpadded row h0-5+k
WC = 132         # slab/XD2 col count
NSLOT = NR * WC  # 9768


def _install_ntff_hook():
    if "antenv.axon_hooks" in sys.modules:
        return
    try:
        import antenv
        from trn_agent_boot.trn_boot import _ntff_profile_via_ctypes
    except Exception:
        return
    mod = types.ModuleType("antenv.axon_hooks")
    _hook = [None]
    mod.set_axon_ntff_profile_hook = lambda h: _hook.__setitem__(0, h)
    mod.get_axon_ntff_profile_hook = lambda: _hook[0]
    sys.modules["antenv.axon_hooks"] = mod
    antenv.axon_hooks = mod
    try:
        mod.set_axon_ntff_profile_hook(
            _ntff_profile_via_ctypes("/opt/axon/libaxon_pjrt.so"))
    except Exception:
        mod.set_axon_ntff_profile_hook(None)


_PROGRAM = None


def _build_program():
    global _PROGRAM
    if _PROGRAM is not None:
        return _PROGRAM
    from contextlib import ExitStack
    import concourse.bass as bass
    import concourse.tile as tile
    from concourse import mybir, bacc

    f32 = mybir.dt.float32
    bf16 = mybir.dt.bfloat16
    i16 = mybir.dt.int16
    i32 = mybir.dt.int32
    A = mybir.AluOpType

    nc = bacc.Bacc(num_swdge_queues=4)
    # ---- I/O ----
    xg_p = nc.declare_dram_parameter("xg", [64, NR * 128], bf16, isOutput=False)
    xhw_p = nc.declare_dram_parameter("xhw", [128, NR * 64], bf16, isOutput=False)
    base2_p = nc.declare_dram_parameter("base2", [128, 64 * 18], f32, isOutput=False)
    xsc_p = nc.declare_dram_parameter("xsc", [128, 2], f32, isOutput=False)
    woff_p = nc.declare_dram_parameter("woff", [64, 9 * 18], bf16, isOutput=False)
    wca_p = nc.declare_dram_parameter("wconv_a", [128, 256], bf16, isOutput=False)
    wcb_p = nc.declare_dram_parameter("wconv_b", [64, 64], bf16, isOutput=False)
    idf_p = nc.declare_dram_parameter("ident_f", [128, 128], f32, isOutput=False)
    idb_p = nc.declare_dram_parameter("ident_b", [128, 128], bf16, isOutput=False)
    out_p = nc.declare_dram_parameter("out", [64, 64 * 128], f32, isOutput=True)

    xd2 = nc.dram_tensor("xd2", [NSLOT, 256], bf16)        # gather source
    gstage = nc.dram_tensor("gstage", [16, 4 * 1152], i16)  # idx staging

    with tile.TileContext(nc) as tc, ExitStack() as ctx:
        consts = ctx.enter_context(tc.tile_pool(name="consts", bufs=1))
        slab = ctx.enter_context(tc.tile_pool(name="slab", bufs=1))

        # ---------- load constants ----------
        base2 = consts.tile([128, 64, 18], f32)
        nc.sync.dma_start(out=base2,
                          in_=base2_p[:, :].rearrange("a (t c) -> a t c", t=64))
        xsc = consts.tile([128, 2], f32)
        nc.sync.dma_start(out=xsc, in_=xsc_p[:, :])
        woff = consts.tile([64, 9, 18], bf16)
        nc.sync.dma_start(out=woff, in_=woff_p[:, :].rearrange("a (t c) -> a t c", t=9))
        wca = consts.tile([128, 256], bf16)
        nc.sync.dma_start(out=wca, in_=wca_p[:, :])
        wcb = consts.tile([64, 64], bf16)
        nc.sync.dma_start(out=wcb, in_=wcb_p[:, :])
        idf = consts.tile([128, 128], f32)
        nc.sync.dma_start(out=idf, in_=idf_p[:, :])
        idb = consts.tile([128, 128], bf16)
        nc.sync.dma_start(out=idb, in_=idb_p[:, :])

        # ---------- phase A: x slab load + XD2 build ----------
        xsb = slab.tile([64, NR, WC], bf16, name="xsb")
        nc.vector.memset(xsb[:, :, 0:1], 0.0)
        nc.vector.memset(xsb[:, :, 129:132], 0.0)
        nc.sync.dma_start(
            out=xsb[:, :, 1:129],
            in_=xg_p[:, :].rearrange("c (r w) -> c r w", r=NR))

        a_ctx = ExitStack()
        apool = a_ctx.enter_context(tc.tile_pool(name="apool", bufs=1))

        # border zero-fill: only slots the gather can read but no row write
        # covers: col0 subs(0,0)/(1,0), col128 subs(0,1)/(1,1), col129 all.
        zb = apool.tile([74, 256], bf16, name="zb")
        nc.vector.memset(zb, 0.0)
        nc.sync.dma_start(
            out=bass.AP(tensor=xd2, offset=0 * 256,
                        ap=[[WC * 256, NR], [128, 2], [1, 64]]),
            in_=zb[:, 0:128].rearrange("r (a b) -> r a b", a=2))
        nc.sync.dma_start(
            out=bass.AP(tensor=xd2, offset=128 * 256 + 64,
                        ap=[[WC * 256, NR], [128, 2], [1, 64]]),
            in_=zb[:, 0:128].rearrange("r (a b) -> r a b", a=2))
        nc.sync.dma_start(
            out=bass.AP(tensor=xd2, offset=129 * 256,
                        ap=[[WC * 256, NR], [1, 256]]),
            in_=zb)

        # host-transposed HWC slab -> write the 4 sub-blocks of XD2
        xhw = slab.tile([128, NR, 64], bf16, name="xhw")
        nc.sync.dma_start(
            out=xhw, in_=xhw_p[:, :].rearrange("w (r c) -> w r c", r=NR))
        # sub (0,0): slot (k, c=w+1) bytes [0,64), rows 0..73
        nc.sync.dma_start(
            out=bass.AP(tensor=xd2, offset=(0 * WC + 1) * 256 + 0,
                        ap=[[256, 128], [WC * 256, NR], [1, 64]]),
            in_=xhw)
        # sub (0,1): slot (k, c=w) bytes [64,128)
        nc.sync.dma_start(
            out=bass.AP(tensor=xd2, offset=(0 * WC + 0) * 256 + 64,
                        ap=[[256, 128], [WC * 256, NR], [1, 64]]),
            in_=xhw)
        # sub (1,0): slot (k-1, c=w+1) bytes [128,192), slot rows 0..72
        nc.sync.dma_start(
            out=bass.AP(tensor=xd2, offset=(0 * WC + 1) * 256 + 128,
                        ap=[[256, 128], [WC * 256, NR - 1], [1, 64]]),
            in_=xhw[:, 1:NR, :])
        # sub (1,1): slot (k-1, c=w) bytes [192,256)
        nc.sync.dma_start(
            out=bass.AP(tensor=xd2, offset=(0 * WC + 0) * 256 + 192,
                        ap=[[256, 128], [WC * 256, NR - 1], [1, 64]]),
            in_=xhw[:, 1:NR, :])
        a_ctx.close()

        # ---------- pools for phases B/C/D ----------
        ps_c = ctx.enter_context(tc.tile_pool(name="ps_c", bufs=2, space="PSUM"))
        ps_t = ctx.enter_context(tc.tile_pool(name="ps_t", bufs=2, space="PSUM"))
        ps_x = ctx.enter_context(tc.tile_pool(name="ps_x", bufs=2, space="PSUM"))
        ps_o = ctx.enter_context(tc.tile_pool(name="ps_o", bufs=2, space="PSUM"))
        bpool = ctx.enter_context(tc.tile_pool(name="bpool", bufs=2))
        cpool = ctx.enter_context(tc.tile_pool(name="cpool", bufs=2))
        tmpp = ctx.enter_context(tc.tile_pool(name="tmpp", bufs=4))
        ipool = ctx.enter_context(tc.tile_pool(name="ipool", bufs=2))
        gpool = ctx.enter_context(tc.tile_pool(name="gpool", bufs=6))
        dpool = ctx.enter_context(tc.tile_pool(name="dpool", bufs=2))
        xopool = ctx.enter_context(tc.tile_pool(name="xopool", bufs=2))
        rpool = ctx.enter_context(tc.tile_pool(name="rpool", bufs=2))
        opool = ctx.enter_context(tc.tile_pool(name="opool", bufs=2))
        jpool = ctx.enter_context(tc.tile_pool(name="jpool", bufs=4))

        # pre-drain XD2-ready onto Pool (gather ISA supports few sem waits)
        j1 = jpool.tile([16, 8], bf16, tag="j1", name="j1")
        nc.sync.dma_start(out=j1[0:1, 0:8], in_=xd2[0:1, 0:8])
        j3 = jpool.tile([16, 8], bf16, tag="j3", name="j3")
        nc.gpsimd.tensor_copy(j3[0:1, 0:4], j1[0:1, 0:4])

        def phase_b(tcn):
            off_sb = bpool.tile([18, 16, 128], f32, tag="offsb")
            for tb in range(4):
                psc = ps_c.tile([18, 512], f32, tag="c")
                for dy in range(3):
                    for dx in range(3):
                        tap = dy * 3 + dx
                        nc.tensor.matmul(
                            psc[:, :],
                            woff[:, tap, :],
                            bass.AP(tensor=xsb.tensor,
                                    offset=xsb.offset
                                    + (tcn * 16 + tb * 4 + dy + 4) * WC + dx,
                                    ap=[xsb.ap[0], [WC, 4], [1, 128]]),
                            start=(tap == 0), stop=(tap == 8))
                nc.any.tensor_copy(
                    off_sb[:, tb * 4:tb * 4 + 4, :],
                    psc[:, :].rearrange("p (r w) -> p r w", r=4))
            pst = ps_t.tile([128, 288], f32, tag="t")
            for j in range(16):
                nc.tensor.transpose(
                    pst[:, j * 18:(j + 1) * 18], off_sb[:, j, :],
                    idf[0:18, 0:18])
            offt = cpool.tile([128, 16, 18], f32, tag="offt")
            nc.any.tensor_copy(
                offt, pst[:, :].rearrange("p (t c) -> p t c", t=16))
            return offt

        def phase_c(tcn, offt):
            P = cpool.tile([128, 16, 18], f32, tag="P")
            nc.vector.tensor_tensor(
                P, offt, base2[:, tcn * 16:(tcn + 1) * 16, :], A.add)
            q_i = tmpp.tile([128, 16, 18], i32, tag="ct", name="qi")
            nc.vector.tensor_copy(q_i, P)
            Qf0 = tmpp.tile([128, 16, 18], f32, tag="ct", name="qf0")
            nc.vector.tensor_copy(Qf0, q_i)
            GT = tmpp.tile([128, 16, 18], f32, tag="ct", name="gt")
            nc.vector.tensor_tensor(GT, Qf0, P, A.is_gt)
            Qf = cpool.tile([128, 16, 18], f32, tag="Qf")
            nc.vector.tensor_tensor(Qf, Qf0, GT, A.subtract)
            FR = tmpp.tile([128, 16, 18], f32, tag="ct", name="fr")
            nc.vector.tensor_tensor(FR, P, Qf, A.subtract)
            INR = tmpp.tile([128, 16, 18], f32, tag="ct", name="inr")
            nc.vector.tensor_scalar(INR[:, :, 0:9], P[:, :, 0:9],
                                    xsc[:, 0:1], None, A.is_ge)
            nc.vector.tensor_scalar(INR[:, :, 9:18], P[:, :, 9:18],
                                    9.0, None, A.is_ge)
            INH = tmpp.tile([128, 16, 18], f32, tag="ct", name="inh")
            nc.vector.tensor_scalar(INH[:, :, 0:9], P[:, :, 0:9],
                                    xsc[:, 1:2], None, A.is_le)
            nc.vector.tensor_scalar(INH[:, :, 9:18], P[:, :, 9:18],
                                    136.0, None, A.is_le)
            nc.vector.tensor_tensor(INR, INR, INH, A.mult)
            FRV = cpool.tile([128, 16, 18], f32, tag="FRV")
            nc.vector.tensor_tensor(FRV, FR, INR, A.mult)
            QC = cpool.tile([128, 16, 18], f32, tag="QC")
            nc.vector.tensor_scalar(QC[:, :, 0:9], Qf[:, :, 0:9],
                                    4.0, 76.0, A.max, A.min)
            nc.vector.tensor_scalar(QC[:, :, 9:18], Qf[:, :, 9:18],
                                    8.0, 137.0, A.max, A.min)
            LINF = cpool.tile([128, 16, 9], f32, tag="LINF")
            nc.vector.tensor_scalar(LINF, QC[:, :, 0:9], 132.0, -536.0,
                                    A.mult, A.add)
            nc.vector.tensor_tensor(LINF, LINF, QC[:, :, 9:18], A.add)
            # n-major int16 idx [128 w, 9 n, 16 t]
            gpre = ipool.tile([128, 9, 16], i16, tag="gpre")
            nc.vector.tensor_copy(
                gpre,
                bass.AP(tensor=LINF.tensor, offset=LINF.offset,
                        ap=[LINF.ap[0], [1, 9], [9, 16]]))
            # relayout via DRAM: gstage[pl, (ph, n, tt)]; the load DMA reads
            # it back in (n, tt, ph) order so sg2 is gather-ready.
            gst_off = tcn * 1152
            for ph in range(8):
                sl = gpre[ph * 16:ph * 16 + 16]
                nc.sync.dma_start(
                    out=bass.AP(tensor=gstage, offset=gst_off + ph * 144,
                                ap=[[4608, 16], [1, 144]]),
                    in_=bass.AP(tensor=sl.tensor, offset=sl.offset,
                                ap=[sl.ap[0], [1, 144]]))
            sg = ipool.tile([128, 8, 144], i16, tag="sg")
            for grp in range(8):
                nc.sync.dma_start(
                    out=sg[grp * 16:(grp + 1) * 16, :, :],
                    in_=bass.AP(tensor=gstage, offset=gst_off,
                                ap=[[4608, 16], [144, 8], [1, 144]]))
            # ph-interleave on GPSIMD: sg2[p, n, tt*8+ph] = sg[p, ph, n*16+tt]
            sg2 = ipool.tile([128, 9, 128], i16, tag="sg2")
            nc.gpsimd.tensor_copy(
                bass.AP(tensor=sg2.tensor, offset=sg2.offset,
                        ap=[sg2.ap[0], [128, 9], [8, 16], [1, 8]]),
                bass.AP(tensor=sg.tensor, offset=sg.offset,
                        ap=[sg.ap[0], [16, 9], [1, 16], [144, 8]]))
            return FRV, sg2

        def issue_gathers(tcn, sg2):
            jA = jpool.tile([16, 8], i16, tag="jA")
            nc.gpsimd.tensor_copy(jA[0:16, 0:4], sg2[0:16, 0, 0:4])
            gs = []
            for n in range(9):
                g = gpool.tile([128, 16, 2, 2, 64], bf16, tag="g")
                nc.gpsimd.dma_gather(
                    out_ap=g.rearrange("p a b c d -> p a (b c d)"),
                    in_ap=xd2[:, :],
                    idxs_ap=sg2[:, n, :],
                    num_idxs=2048,
                    num_idxs_reg=2048,
                    elem_size=256,
                    single_packet=False,
                    queue_num=n % 4,
                )
                gs.append(g)
            return gs

        def combine_n(st, n):
            g = st["gs"][n]
            FRV = st["FRV"]
            xoff = st["xoff"]
            D = dpool.tile([128, 16, 2, 64], bf16, tag="D")
            nc.vector.tensor_tensor(
                D, g[:, :, 1, :, :], g[:, :, 0, :, :], A.subtract)
            H = dpool.tile([128, 16, 2, 64], bf16, tag="H")
            for tt in range(16):
                nc.vector.scalar_tensor_tensor(
                    H[:, tt, :, :], D[:, tt, :, :],
                    FRV[:, tt, n:n + 1], g[:, tt, 0, :, :],
                    A.mult, A.add)
            D2 = dpool.tile([128, 16, 64], bf16, tag="D2")
            nc.vector.tensor_tensor(
                D2, H[:, :, 1, :], H[:, :, 0, :], A.subtract)
            for tt in range(16):
                nc.scalar.mul(D2[:, tt, :], D2[:, tt, :],
                              FRV[:, tt, 9 + n:10 + n])
            nc.vector.tensor_tensor(
                xoff[:, :, n, :], H[:, :, 0, :], D2, A.add)

        def conv_out(st):
            tcn = st["tcn"]
            xoff = st["xoff"]
            outb = opool.tile([64, 4, 512], f32, tag="outb")
            for g4 in range(4):
                rhs = rpool.tile([128, 5, 512], bf16, tag="rhs")
                for jc in range(4):
                    psx = ps_x.tile([128, 4, 128], bf16, tag="x")
                    for ti in range(4):
                        tt = g4 * 4 + ti
                        nc.tensor.transpose(
                            psx[:, ti, :],
                            xoff[:, tt, 2 * jc:2 * jc + 2, :].rearrange(
                                "p a b -> p (a b)"),
                            idb)
                    nc.any.tensor_copy(
                        rhs[:, jc, :].rearrange("p (a b) -> p a b", a=4), psx)
                psx9 = ps_x.tile([128, 4, 128], bf16, tag="x")
                for ti in range(4):
                    tt = g4 * 4 + ti
                    nc.tensor.transpose(
                        psx9[0:64, ti, :], xoff[:, tt, 8, :], idb)
                nc.any.tensor_copy(
                    rhs[0:64, 4, :].rearrange("p (a b) -> p a b", a=4),
                    psx9[0:64, :, :])
                pso = ps_o.tile([64, 512], f32, tag="o")
                for jc in range(4):
                    nc.tensor.matmul(pso, wca[:, jc * 64:(jc + 1) * 64],
                                     rhs[:, jc, :], start=(jc == 0), stop=False)
                nc.tensor.matmul(pso, wcb, rhs[0:64, 4, :],
                                 start=False, stop=True)
                nc.any.tensor_copy(outb[:, g4, :], pso)
            # ACT-queue DMA: keeps the sync queue free for idx staging
            nc.scalar.dma_start(
                out=out_p[:, tcn * 2048:(tcn + 1) * 2048],
                in_=outb.rearrange("c a b -> c (a b)"))

        prev = None
        for tcn in range(4):
            offt = phase_b(tcn)
            if prev is not None:
                for n in range(3):
                    combine_n(prev, n)
            FRV, sg2 = phase_c(tcn, offt)
            gs = issue_gathers(tcn, sg2)
            xoff = xopool.tile([128, 16, 9, 64], bf16, tag="xoff", name="xoff")
            cur = {"tcn": tcn, "gs": gs, "FRV": FRV, "xoff": xoff}
            if prev is not None:
                for n in range(3, 9):
                    combine_n(prev, n)
                conv_out(prev)
            prev = cur
        for n in range(9):
            combine_n(prev, n)
        conv_out(prev)

    nc.finalize()
    _PROGRAM = nc
    return nc


def _host_consts(W_off, b_off, W_conv):
    idxr = np.concatenate([np.arange(0, 18, 2), np.arange(1, 18, 2)])
    W_off_r = W_off[idxr]            # (18, 64, 3, 3)
    b_off_r = b_off[idxr]            # (18,)
    woff = np.ascontiguousarray(
        W_off_r.transpose(2, 3, 1, 0).reshape(9, 64, 18).transpose(1, 0, 2)
    ).reshape(64, 9 * 18).astype(BF16)
    # base2 [128 w, 64 t, 18]
    nidx = np.arange(9)
    pnx = (nidx // 3) - 1
    pny = (nidx % 3) - 1
    tt = np.arange(64)
    ww = np.arange(128)
    base2 = np.zeros((128, 64, 18), np.float32)
    base2[:, :, 0:9] = tt[None, :, None] + 9 + pnx[None, None, :] + \
        b_off_r[None, None, 0:9]
    base2[:, :, 9:18] = ww[:, None, None] + 9 + pny[None, None, :] + \
        b_off_r[None, None, 9:18]
    base2 = base2.reshape(128, 64 * 18)
    # final conv weights
    Wmat = W_conv.reshape(64, 64, 9).transpose(0, 2, 1)   # (co, n, ci)
    wca = np.zeros((128, 256), np.float32)
    for jc in range(4):
        for dn in range(2):
            wca[dn * 64:(dn + 1) * 64, jc * 64:(jc + 1) * 64] = \
                Wmat[:, 2 * jc + dn, :].T
    wcb = np.ascontiguousarray(Wmat[:, 8, :].T)           # (ci, co)
    return {
        "woff": woff,
        "base2": base2,
        "wconv_a": wca.astype(BF16),
        "wconv_b": wcb.astype(BF16),
        "ident_f": np.eye(128, dtype=np.float32),
        "ident_b": np.eye(128, dtype=np.float32).astype(BF16),
    }


def _per_core_inputs(x, consts, s, half):
    h0 = 64 * half
    xs = x[s]                                    # (64, 128, 128)
    xgs = np.zeros((64, NR, 128), BF16)
    lo = h0 - 5                                  # unpadded row of slab row 0
    r0 = max(0, lo)
    r1 = min(128, lo + NR)
    xgs[:, r0 - lo:r1 - lo, :] = xs[:, r0:r1, :].astype(BF16)
    xhw = np.ascontiguousarray(xgs.transpose(2, 1, 0))   # (128 w, 74 r, 64 c)
    xsc = np.zeros((128, 2), np.float32)
    xsc[:, 0] = 9 - h0                           # x mask lo
    xsc[:, 1] = 136 - h0                         # x mask hi
    return {
        "xg": xgs.reshape(64, NR * 128),
        "xhw": xhw.reshape(128, NR * 64),
        "xsc": xsc,
        **consts,
    }


def kernel(x, W_off, b_off, W_conv):
    _install_ntff_hook()
    # the bass kernel must run on the axon trn2 backend; undo any cpu pin
    import os
    if os.environ.get("JAX_PLATFORMS", "") == "cpu":
        try:
            import jax
            jax.config.update("jax_platforms", None)
            os.environ.pop("JAX_PLATFORMS", None)
        except Exception:
            pass
    x = np.asarray(x, np.float32)
    W_off = np.asarray(W_off, np.float32)
    b_off = np.asarray(b_off, np.float32)
    W_conv = np.asarray(W_conv, np.float32)

    from concourse.bass_utils import run_bass_kernel_spmd
    nc = _build_program()
    consts = _host_consts(W_off, b_off, W_conv)
    in_maps = [
        _per_core_inputs(x, consts, core // 2, core % 2) for core in range(NCORES)
    ]
    res = run_bass_kernel_spmd(nc, in_maps, list(range(NCORES)))
    out = np.empty((4, 64, 128, 128), np.float32)
    for core in range(NCORES):
        s, half = core // 2, core % 2
        out[s, :, 64 * half:64 * half + 64, :] = \
            res.results[core]["out"].reshape(64, 64, 128)
    return out


# revision 17
# speedup vs baseline: 4.1174x; 4.1174x over previous
"""Trainium2 Bass kernel for DeformConv2D (b=4, c=64, H=W=128, ks=3).

Sharding: 8 cores = (sample s = core//2) x (row-half = core%2). Each core
computes output rows [64*half, 64*half+64) of its sample.

v3 dataflow (per core), software-pipelined over 4 t-chunks of 16 rows:
  A. Load a 74-row bf16 halo slab of x (CHW) into SBUF; build XD2 in DRAM:
     [74*132 slots, 512B] where slot (k,c) = 2rows x 2cols x 64ch -- ONE
     gather descriptor fetches a full bilinear corner block per sample.
  B. Offset conv on PE (bf16, 9 taps, K=64); PE-transpose to [128w,16t,18].
  C. DVE coordinate pipeline -> masked fracs + linear slot idx; idx
     relayout to the gather's wrapped-16 layout via DRAM staging (the
     ph-interleave is folded into the staging-load DMA access pattern).
  D. 9 dma_gathers per chunk (2048 idxs, 512B elems, 6-deep buffer ring);
     combine = lerp-of-lerp: DVE batched corner deltas + fused row madds,
     ACT engine does the column-stage multiplies; PE transpose + final
     conv as 512-wide accumulating matmuls. Out-DMA rides the ACT queue
     so it never blocks the sync-queue idx staging.
  Chunk t+1's B/C phases are issued before chunk t's combine so the Pool
  engine's gather stream never starves.
"""
import sys
import types
import numpy as np
import ml_dtypes

sys.path.insert(0, "/opt/trn_rl_repo")

BF16 = ml_dtypes.bfloat16
NCORES = 8
NR = 74          # slab rows (local): row k <-> unpadded row h0-5+k
WC = 132         # slab/XD2 col count
NSLOT = NR * WC  # 9768


def _install_ntff_hook():
    if "antenv.axon_hooks" in sys.modules:
        return
    try:
        import antenv
        from trn_agent_boot.trn_boot import _ntff_profile_via_ctypes
    except Exception:
        return
    mod = types.ModuleType("antenv.axon_hooks")
    _hook = [None]
    mod.set_axon_ntff_profile_hook = lambda h: _hook.__setitem__(0, h)
    mod.get_axon_ntff_profile_hook = lambda: _hook[0]
    sys.modules["antenv.axon_hooks"] = mod
    antenv.axon_hooks = mod
    try:
        mod.set_axon_ntff_profile_hook(
            _ntff_profile_via_ctypes("/opt/axon/libaxon_pjrt.so"))
    except Exception:
        mod.set_axon_ntff_profile_hook(None)


_PROGRAM = None


def _build_program():
    global _PROGRAM
    if _PROGRAM is not None:
        return _PROGRAM
    from contextlib import ExitStack
    import concourse.bass as bass
    import concourse.tile as tile
    from concourse import mybir, bacc

    f32 = mybir.dt.float32
    bf16 = mybir.dt.bfloat16
    i16 = mybir.dt.int16
    i32 = mybir.dt.int32
    A = mybir.AluOpType

    nc = bacc.Bacc(num_swdge_queues=4)
    # ---- I/O ----
    xg_p = nc.declare_dram_parameter("xg", [64, NR * 128], bf16, isOutput=False)
    xhw_p = nc.declare_dram_parameter("xhw", [128, NR * 64], bf16, isOutput=False)
    base2_p = nc.declare_dram_parameter("base2", [128, 64 * 18], f32, isOutput=False)
    xsc_p = nc.declare_dram_parameter("xsc", [128, 2], f32, isOutput=False)
    woff_p = nc.declare_dram_parameter("woff", [64, 9 * 18], bf16, isOutput=False)
    wca_p = nc.declare_dram_parameter("wconv_a", [128, 256], bf16, isOutput=False)
    wcb_p = nc.declare_dram_parameter("wconv_b", [64, 64], bf16, isOutput=False)
    idf_p = nc.declare_dram_parameter("ident_f", [128, 128], f32, isOutput=False)
    idb_p = nc.declare_dram_parameter("ident_b", [128, 128], bf16, isOutput=False)
    out_p = nc.declare_dram_parameter("out", [64, 64 * 128], f32, isOutput=True)

    xd2 = nc.dram_tensor("xd2", [NSLOT, 256], bf16)        # gather source
    gstage = nc.dram_tensor("gstage", [16, 4 * 1152], i16)  # idx staging

    with tile.TileContext(nc) as tc, ExitStack() as ctx:
        consts = ctx.enter_context(tc.tile_pool(name="consts", bufs=1))
        slab = ctx.enter_context(tc.tile_pool(name="slab", bufs=1))

        # ---------- load constants ----------
        base2 = consts.tile([128, 64, 18], f32)
        nc.sync.dma_start(out=base2,
                          in_=base2_p[:, :].rearrange("a (t c) -> a t c", t=64))
        xsc = consts.tile([128, 2], f32)
        nc.sync.dma_start(out=xsc, in_=xsc_p[:, :])
        woff = consts.tile([64, 9, 18], bf16)
        nc.sync.dma_start(out=woff, in_=woff_p[:, :].rearrange("a (t c) -> a t c", t=9))
        wca = consts.tile([128, 256], bf16)
        nc.sync.dma_start(out=wca, in_=wca_p[:, :])
        wcb = consts.tile([64, 64], bf16)
        nc.sync.dma_start(out=wcb, in_=wcb_p[:, :])
        idf = consts.tile([128, 128], f32)
        nc.sync.dma_start(out=idf, in_=idf_p[:, :])
        idb = consts.tile([128, 128], bf16)
        nc.sync.dma_start(out=idb, in_=idb_p[:, :])

        # ---------- phase A: x slab load + XD2 build ----------
        xsb = slab.tile([64, NR, WC], bf16, name="xsb")
        nc.vector.memset(xsb[:, :, 0:1], 0.0)
        nc.vector.memset(xsb[:, :, 129:132], 0.0)
        nc.sync.dma_start(
            out=xsb[:, :, 1:129],
            in_=xg_p[:, :].rearrange("c (r w) -> c r w", r=NR))

        a_ctx = ExitStack()
        apool = a_ctx.enter_context(tc.tile_pool(name="apool", bufs=1))

        # border zero-fill: only slots the gather can read but no row write
        # covers: col0 subs(0,0)/(1,0), col128 subs(0,1)/(1,1), col129 all.
        zb = apool.tile([74, 256], bf16, name="zb")
        nc.vector.memset(zb, 0.0)
        nc.sync.dma_start(
            out=bass.AP(tensor=xd2, offset=0 * 256,
                        ap=[[WC * 256, NR], [128, 2], [1, 64]]),
            in_=zb[:, 0:128].rearrange("r (a b) -> r a b", a=2))
        nc.sync.dma_start(
            out=bass.AP(tensor=xd2, offset=128 * 256 + 64,
                        ap=[[WC * 256, NR], [128, 2], [1, 64]]),
            in_=zb[:, 0:128].rearrange("r (a b) -> r a b", a=2))
        nc.sync.dma_start(
            out=bass.AP(tensor=xd2, offset=129 * 256,
                        ap=[[WC * 256, NR], [1, 256]]),
            in_=zb)

        # host-transposed HWC slab -> write the 4 sub-blocks of XD2
        xhw = slab.tile([128, NR, 64], bf16, name="xhw")
        nc.sync.dma_start(
            out=xhw, in_=xhw_p[:, :].rearrange("w (r c) -> w r c", r=NR))
        # sub (0,0): slot (k, c=w+1) bytes [0,64), rows 0..73
        nc.sync.dma_start(
            out=bass.AP(tensor=xd2, offset=(0 * WC + 1) * 256 + 0,
                        ap=[[256, 128], [WC * 256, NR], [1, 64]]),
            in_=xhw)
        # sub (0,1): slot (k, c=w) bytes [64,128)
        nc.sync.dma_start(
            out=bass.AP(tensor=xd2, offset=(0 * WC + 0) * 256 + 64,
                        ap=[[256, 128], [WC * 256, NR], [1, 64]]),
            in_=xhw)
        # sub (1,0): slot (k-1, c=w+1) bytes [128,192), slot rows 0..72
        nc.sync.dma_start(
            out=bass.AP(tensor=xd2, offset=(0 * WC + 1) * 256 + 128,
                        ap=[[256, 128], [WC * 256, NR - 1], [1, 64]]),
            in_=xhw[:, 1:NR, :])
        # sub (1,1): slot (k-1, c=w) bytes [192,256)
        nc.sync.dma_start(
            out=bass.AP(tensor=xd2, offset=(0 * WC + 0) * 256 + 192,
                        ap=[[256, 128], [WC * 256, NR - 1], [1, 64]]),
            in_=xhw[:, 1:NR, :])
        a_ctx.close()

        # ---------- pools for phases B/C/D ----------
        ps_c = ctx.enter_context(tc.tile_pool(name="ps_c", bufs=2, space="PSUM"))
        ps_t = ctx.enter_context(tc.tile_pool(name="ps_t", bufs=2, space="PSUM"))
        ps_x = ctx.enter_context(tc.tile_pool(name="ps_x", bufs=2, space="PSUM"))
        ps_o = ctx.enter_context(tc.tile_pool(name="ps_o", bufs=2, space="PSUM"))
        bpool = ctx.enter_context(tc.tile_pool(name="bpool", bufs=2))
        cpool = ctx.enter_context(tc.tile_pool(name="cpool", bufs=2))
        tmpp = ctx.enter_context(tc.tile_pool(name="tmpp", bufs=4))
        ipool = ctx.enter_context(tc.tile_pool(name="ipool", bufs=2))
        gpool = ctx.enter_context(tc.tile_pool(name="gpool", bufs=6))
        dpool = ctx.enter_context(tc.tile_pool(name="dpool", bufs=2))
        xopool = ctx.enter_context(tc.tile_pool(name="xopool", bufs=2))
        rpool = ctx.enter_context(tc.tile_pool(name="rpool", bufs=2))
        opool = ctx.enter_context(tc.tile_pool(name="opool", bufs=2))
        jpool = ctx.enter_context(tc.tile_pool(name="jpool", bufs=4))

        # pre-drain XD2-ready onto Pool (gather ISA supports few sem waits)
        j1 = jpool.tile([16, 8], bf16, tag="j1", name="j1")
        nc.sync.dma_start(out=j1[0:1, 0:8], in_=xd2[0:1, 0:8])
        j3 = jpool.tile([16, 8], bf16, tag="j3", name="j3")
        nc.gpsimd.tensor_copy(j3[0:1, 0:4], j1[0:1, 0:4])

        def phase_b(tcn):
            off_sb = bpool.tile([18, 16, 128], f32, tag="offsb")
            for tb in range(4):
                psc = ps_c.tile([18, 512], f32, tag="c")
                for dy in range(3):
                    for dx in range(3):
                        tap = dy * 3 + dx
                        nc.tensor.matmul(
                            psc[:, :],
                            woff[:, tap, :],
                            bass.AP(tensor=xsb.tensor,
                                    offset=xsb.offset
                                    + (tcn * 16 + tb * 4 + dy + 4) * WC + dx,
                                    ap=[xsb.ap[0], [WC, 4], [1, 128]]),
                            start=(tap == 0), stop=(tap == 8))
                nc.any.tensor_copy(
                    off_sb[:, tb * 4:tb * 4 + 4, :],
                    psc[:, :].rearrange("p (r w) -> p r w", r=4))
            pst = ps_t.tile([128, 288], f32, tag="t")
            for j in range(16):
                nc.tensor.transpose(
                    pst[:, j * 18:(j + 1) * 18], off_sb[:, j, :],
                    idf[0:18, 0:18])
            offt = cpool.tile([128, 16, 18], f32, tag="offt")
            nc.any.tensor_copy(
                offt, pst[:, :].rearrange("p (t c) -> p t c", t=16))
            return offt

        def phase_c(tcn, offt):
            P = cpool.tile([128, 16, 18], f32, tag="P")
            nc.vector.tensor_tensor(
                P, offt, base2[:, tcn * 16:(tcn + 1) * 16, :], A.add)
            q_i = tmpp.tile([128, 16, 18], i32, tag="ct", name="qi")
            nc.vector.tensor_copy(q_i, P)
            Qf0 = tmpp.tile([128, 16, 18], f32, tag="ct", name="qf0")
            nc.vector.tensor_copy(Qf0, q_i)
            GT = tmpp.tile([128, 16, 18], f32, tag="ct", name="gt")
            nc.vector.tensor_tensor(GT, Qf0, P, A.is_gt)
            Qf = cpool.tile([128, 16, 18], f32, tag="Qf")
            nc.vector.tensor_tensor(Qf, Qf0, GT, A.subtract)
            FR = tmpp.tile([128, 16, 18], f32, tag="ct", name="fr")
            nc.vector.tensor_tensor(FR, P, Qf, A.subtract)
            INR = tmpp.tile([128, 16, 18], f32, tag="ct", name="inr")
            nc.vector.tensor_scalar(INR[:, :, 0:9], P[:, :, 0:9],
                                    xsc[:, 0:1], None, A.is_ge)
            nc.vector.tensor_scalar(INR[:, :, 9:18], P[:, :, 9:18],
                                    9.0, None, A.is_ge)
            INH = tmpp.tile([128, 16, 18], f32, tag="ct", name="inh")
            nc.vector.tensor_scalar(INH[:, :, 0:9], P[:, :, 0:9],
                                    xsc[:, 1:2], None, A.is_le)
            nc.vector.tensor_scalar(INH[:, :, 9:18], P[:, :, 9:18],
                                    136.0, None, A.is_le)
            nc.vector.tensor_tensor(INR, INR, INH, A.mult)
            FRV = cpool.tile([128, 16, 18], f32, tag="FRV")
            nc.vector.tensor_tensor(FRV, FR, INR, A.mult)
            QC = cpool.tile([128, 16, 18], f32, tag="QC")
            nc.vector.tensor_scalar(QC[:, :, 0:9], Qf[:, :, 0:9],
                                    4.0, 76.0, A.max, A.min)
            nc.vector.tensor_scalar(QC[:, :, 9:18], Qf[:, :, 9:18],
                                    8.0, 137.0, A.max, A.min)
            LINF = cpool.tile([128, 16, 9], f32, tag="LINF")
            nc.vector.tensor_scalar(LINF, QC[:, :, 0:9], 132.0, -536.0,
                                    A.mult, A.add)
            nc.vector.tensor_tensor(LINF, LINF, QC[:, :, 9:18], A.add)
            # n-major int16 idx [128 w, 9 n, 16 t]
            gpre = ipool.tile([128, 9, 16], i16, tag="gpre")
            nc.vector.tensor_copy(
                gpre,
                bass.AP(tensor=LINF.tensor, offset=LINF.offset,
                        ap=[LINF.ap[0], [1, 9], [9, 16]]))
            # relayout via DRAM: gstage[pl, (ph, n, tt)]; the load DMA reads
            # it back in (n, tt, ph) order so sg2 is gather-ready.
            gst_off = tcn * 1152
            for ph in range(8):
                sl = gpre[ph * 16:ph * 16 + 16]
                nc.sync.dma_start(
                    out=bass.AP(tensor=gstage, offset=gst_off + ph * 144,
                                ap=[[4608, 16], [1, 144]]),
                    in_=bass.AP(tensor=sl.tensor, offset=sl.offset,
                                ap=[sl.ap[0], [1, 144]]))
            sg = ipool.tile([128, 8, 144], i16, tag="sg")
            for grp in range(8):
                nc.sync.dma_start(
                    out=sg[grp * 16:(grp + 1) * 16, :, :],
                    in_=bass.AP(tensor=gstage, offset=gst_off,
                                ap=[[4608, 16], [144, 8], [1, 144]]))
            # ph-interleave on GPSIMD: sg2[p, n, tt*8+ph] = sg[p, ph, n*16+tt]
            sg2 = ipool.tile([128, 9, 128], i16, tag="sg2")
            nc.gpsimd.tensor_copy(
                bass.AP(tensor=sg2.tensor, offset=sg2.offset,
                        ap=[sg2.ap[0], [128, 9], [8, 16], [1, 8]]),
                bass.AP(tensor=sg.tensor, offset=sg.offset,
                        ap=[sg.ap[0], [16, 9], [1, 16], [144, 8]]))
            return FRV, sg2

        def issue_gathers(tcn, sg2):
            gs = []
            for n in range(9):
                g = gpool.tile([128, 16, 2, 2, 64], bf16, tag="g")
                nc.gpsimd.dma_gather(
                    out_ap=g.rearrange("p a b c d -> p a (b c d)"),
                    in_ap=xd2[:, :],
                    idxs_ap=sg2[:, n, :],
                    num_idxs=2048,
                    num_idxs_reg=2048,
                    elem_size=256,
                    single_packet=False,
                    queue_num=(n + tcn) % 4,
                )
                gs.append(g)
            return gs

        def combine_n(st, n):
            g = st["gs"][n]
            FRV = st["FRV"]
            xoff = st["xoff"]
            D = dpool.tile([128, 16, 2, 64], bf16, tag="D")
            nc.vector.tensor_tensor(
                D, g[:, :, 1, :, :], g[:, :, 0, :, :], A.subtract)
            H = dpool.tile([128, 16, 2, 64], bf16, tag="H")
            for tt in range(16):
                nc.vector.scalar_tensor_tensor(
                    H[:, tt, :, :], D[:, tt, :, :],
                    FRV[:, tt, n:n + 1], g[:, tt, 0, :, :],
                    A.mult, A.add)
            D2 = dpool.tile([128, 16, 64], bf16, tag="D2")
            nc.vector.tensor_tensor(
                D2, H[:, :, 1, :], H[:, :, 0, :], A.subtract)
            for tt in range(16):
                nc.scalar.mul(D2[:, tt, :], D2[:, tt, :],
                              FRV[:, tt, 9 + n:10 + n])
            nc.vector.tensor_tensor(
                xoff[:, :, n, :], H[:, :, 0, :], D2, A.add)

        def conv_out(st):
            tcn = st["tcn"]
            xoff = st["xoff"]
            outb = opool.tile([64, 4, 512], f32, tag="outb")
            for g4 in range(4):
                rhs = rpool.tile([128, 5, 512], bf16, tag="rhs")
                for jc in range(4):
                    psx = ps_x.tile([128, 4, 128], bf16, tag="x")
                    for ti in range(4):
                        tt = g4 * 4 + ti
                        nc.tensor.transpose(
                            psx[:, ti, :],
                            xoff[:, tt, 2 * jc:2 * jc + 2, :].rearrange(
                                "p a b -> p (a b)"),
                            idb)
                    nc.any.tensor_copy(
                        rhs[:, jc, :].rearrange("p (a b) -> p a b", a=4), psx)
                psx9 = ps_x.tile([128, 4, 128], bf16, tag="x")
                for ti in range(4):
                    tt = g4 * 4 + ti
                    nc.tensor.transpose(
                        psx9[0:64, ti, :], xoff[:, tt, 8, :], idb)
                nc.any.tensor_copy(
                    rhs[0:64, 4, :].rearrange("p (a b) -> p a b", a=4),
                    psx9[0:64, :, :])
                pso = ps_o.tile([64, 512], f32, tag="o")
                for jc in range(4):
                    nc.tensor.matmul(pso, wca[:, jc * 64:(jc + 1) * 64],
                                     rhs[:, jc, :], start=(jc == 0), stop=False)
                nc.tensor.matmul(pso, wcb, rhs[0:64, 4, :],
                                 start=False, stop=True)
                nc.any.tensor_copy(outb[:, g4, :], pso)
            # ACT-queue DMA: keeps the sync queue free for idx staging
            nc.scalar.dma_start(
                out=out_p[:, tcn * 2048:(tcn + 1) * 2048],
                in_=outb.rearrange("c a b -> c (a b)"))

        prev = None
        for tcn in range(4):
            offt = phase_b(tcn)
            FRV, sg2 = phase_c(tcn, offt)
            if prev is not None:
                for n in range(3):
                    combine_n(prev, n)
            gs = issue_gathers(tcn, sg2)
            xoff = xopool.tile([128, 16, 9, 64], bf16, tag="xoff", name="xoff")
            cur = {"tcn": tcn, "gs": gs, "FRV": FRV, "xoff": xoff}
            if prev is not None:
                for n in range(3, 9):
                    combine_n(prev, n)
                conv_out(prev)
            prev = cur
        for n in range(9):
            combine_n(prev, n)
        conv_out(prev)

    nc.finalize()
    _PROGRAM = nc
    return nc


def _host_consts(W_off, b_off, W_conv):
    idxr = np.concatenate([np.arange(0, 18, 2), np.arange(1, 18, 2)])
    W_off_r = W_off[idxr]            # (18, 64, 3, 3)
    b_off_r = b_off[idxr]            # (18,)
    woff = np.ascontiguousarray(
        W_off_r.transpose(2, 3, 1, 0).reshape(9, 64, 18).transpose(1, 0, 2)
    ).reshape(64, 9 * 18).astype(BF16)
    # base2 [128 w, 64 t, 18]
    nidx = np.arange(9)
    pnx = (nidx // 3) - 1
    pny = (nidx % 3) - 1
    tt = np.arange(64)
    ww = np.arange(128)
    base2 = np.zeros((128, 64, 18), np.float32)
    base2[:, :, 0:9] = tt[None, :, None] + 9 + pnx[None, None, :] + \
        b_off_r[None, None, 0:9]
    base2[:, :, 9:18] = ww[:, None, None] + 9 + pny[None, None, :] + \
        b_off_r[None, None, 9:18]
    base2 = base2.reshape(128, 64 * 18)
    # final conv weights
    Wmat = W_conv.reshape(64, 64, 9).transpose(0, 2, 1)   # (co, n, ci)
    wca = np.zeros((128, 256), np.float32)
    for jc in range(4):
        for dn in range(2):
            wca[dn * 64:(dn + 1) * 64, jc * 64:(jc + 1) * 64] = \
                Wmat[:, 2 * jc + dn, :].T
    wcb = np.ascontiguousarray(Wmat[:, 8, :].T)           # (ci, co)
    return {
        "woff": woff,
        "base2": base2,
        "wconv_a": wca.astype(BF16),
        "wconv_b": wcb.astype(BF16),
        "ident_f": np.eye(128, dtype=np.float32),
        "ident_b": np.eye(128, dtype=np.float32).astype(BF16),
    }


def _per_core_inputs(x, consts, s, half):
    h0 = 64 * half
    xs = x[s]                                    # (64, 128, 128)
    xgs = np.zeros((64, NR, 128), BF16)
    lo = h0 - 5                                  # unpadded row of slab row 0
    r0 = max(0, lo)
    r1 = min(128, lo + NR)
    xgs[:, r0 - lo:r1 - lo, :] = xs[:, r0:r1, :].astype(BF16)
    xhw = np.ascontiguousarray(xgs.transpose(2, 1, 0))   # (128 w, 74 r, 64 c)
    xsc = np.zeros((128, 2), np.float32)
    xsc[:, 0] = 9 - h0                           # x mask lo
    xsc[:, 1] = 136 - h0                         # x mask hi
    return {
        "xg": xgs.reshape(64, NR * 128),
        "xhw": xhw.reshape(128, NR * 64),
        "xsc": xsc,
        **consts,
    }


def kernel(x, W_off, b_off, W_conv):
    _install_ntff_hook()
    # the bass kernel must run on the axon trn2 backend; undo any cpu pin
    import os
    if os.environ.get("JAX_PLATFORMS", "") == "cpu":
        try:
            import jax
            jax.config.update("jax_platforms", None)
            os.environ.pop("JAX_PLATFORMS", None)
        except Exception:
            pass
    x = np.asarray(x, np.float32)
    W_off = np.asarray(W_off, np.float32)
    b_off = np.asarray(b_off, np.float32)
    W_conv = np.asarray(W_conv, np.float32)

    from concourse.bass_utils import run_bass_kernel_spmd
    nc = _build_program()
    consts = _host_consts(W_off, b_off, W_conv)
    in_maps = [
        _per_core_inputs(x, consts, core // 2, core % 2) for core in range(NCORES)
    ]
    res = run_bass_kernel_spmd(nc, in_maps, list(range(NCORES)))
    out = np.empty((4, 64, 128, 128), np.float32)
    for core in range(NCORES):
        s, half = core // 2, core % 2
        out[s, :, 64 * half:64 * half + 64, :] = \
            res.results[core]["out"].reshape(64, 64, 128)
    return out


# revision 18
# speedup vs baseline: 4.2549x; 1.0334x over previous
"""Trainium2 Bass kernel for DeformConv2D (b=4, c=64, H=W=128, ks=3).

Sharding: 8 cores = (sample s = core//2) x (row-half = core%2). Each core
computes output rows [64*half, 64*half+64) of its sample.

v3 dataflow (per core), software-pipelined over 4 t-chunks of 16 rows:
  A. Load a 74-row bf16 halo slab of x (CHW) into SBUF; build XD2 in DRAM:
     [74*132 slots, 512B] where slot (k,c) = 2rows x 2cols x 64ch -- ONE
     gather descriptor fetches a full bilinear corner block per sample.
  B. Offset conv on PE (bf16, 9 taps, K=64); PE-transpose to [128w,16t,18].
  C. DVE coordinate pipeline -> masked fracs + linear slot idx; idx
     relayout to the gather's wrapped-16 layout via DRAM staging (the
     ph-interleave is folded into the staging-load DMA access pattern).
  D. 9 dma_gathers per chunk (2048 idxs, 512B elems, 6-deep buffer ring);
     combine = lerp-of-lerp: DVE batched corner deltas + fused row madds,
     ACT engine does the column-stage multiplies; PE transpose + final
     conv as 512-wide accumulating matmuls. Out-DMA rides the ACT queue
     so it never blocks the sync-queue idx staging.
  Chunk t+1's B/C phases are issued before chunk t's combine so the Pool
  engine's gather stream never starves.
"""
import sys
import types
import numpy as np
import ml_dtypes

sys.path.insert(0, "/opt/trn_rl_repo")

BF16 = ml_dtypes.bfloat16
NCORES = 8
NR = 74          # slab rows (local): row k <-> unpadded row h0-5+k
WC = 132         # slab/XD2 col count
NSLOT = NR * WC  # 9768


def _install_ntff_hook():
    if "antenv.axon_hooks" in sys.modules:
        return
    try:
        import antenv
        from trn_agent_boot.trn_boot import _ntff_profile_via_ctypes
    except Exception:
        return
    mod = types.ModuleType("antenv.axon_hooks")
    _hook = [None]
    mod.set_axon_ntff_profile_hook = lambda h: _hook.__setitem__(0, h)
    mod.get_axon_ntff_profile_hook = lambda: _hook[0]
    sys.modules["antenv.axon_hooks"] = mod
    antenv.axon_hooks = mod
    try:
        mod.set_axon_ntff_profile_hook(
            _ntff_profile_via_ctypes("/opt/axon/libaxon_pjrt.so"))
    except Exception:
        mod.set_axon_ntff_profile_hook(None)


_PROGRAM = None


def _build_program():
    global _PROGRAM
    if _PROGRAM is not None:
        return _PROGRAM
    from contextlib import ExitStack
    import concourse.bass as bass
    import concourse.tile as tile
    from concourse import mybir, bacc

    f32 = mybir.dt.float32
    bf16 = mybir.dt.bfloat16
    i16 = mybir.dt.int16
    i32 = mybir.dt.int32
    A = mybir.AluOpType

    nc = bacc.Bacc(num_swdge_queues=4)
    # ---- I/O ----
    xg_p = nc.declare_dram_parameter("xg", [64, NR * 128], bf16, isOutput=False)
    xhw_p = nc.declare_dram_parameter("xhw", [128, NR * 64], bf16, isOutput=False)
    base2_p = nc.declare_dram_parameter("base2", [128, 64 * 18], f32, isOutput=False)
    xsc_p = nc.declare_dram_parameter("xsc", [128, 2], f32, isOutput=False)
    woff_p = nc.declare_dram_parameter("woff", [64, 9 * 18], bf16, isOutput=False)
    wca_p = nc.declare_dram_parameter("wconv_a", [128, 256], bf16, isOutput=False)
    wcb_p = nc.declare_dram_parameter("wconv_b", [64, 64], bf16, isOutput=False)
    idf_p = nc.declare_dram_parameter("ident_f", [128, 128], f32, isOutput=False)
    idb_p = nc.declare_dram_parameter("ident_b", [128, 128], bf16, isOutput=False)
    out_p = nc.declare_dram_parameter("out", [64, 64 * 128], f32, isOutput=True)

    xd2 = nc.dram_tensor("xd2", [NSLOT, 256], bf16)        # gather source
    gstage = nc.dram_tensor("gstage", [16, 4 * 1152], i16)  # idx staging

    with tile.TileContext(nc) as tc, ExitStack() as ctx:
        consts = ctx.enter_context(tc.tile_pool(name="consts", bufs=1))
        slab = ctx.enter_context(tc.tile_pool(name="slab", bufs=1))

        # ---------- load constants ----------
        base2 = consts.tile([128, 64, 18], f32)
        nc.sync.dma_start(out=base2,
                          in_=base2_p[:, :].rearrange("a (t c) -> a t c", t=64))
        xsc = consts.tile([128, 2], f32)
        nc.sync.dma_start(out=xsc, in_=xsc_p[:, :])
        woff = consts.tile([64, 9, 18], bf16)
        nc.sync.dma_start(out=woff, in_=woff_p[:, :].rearrange("a (t c) -> a t c", t=9))
        wca = consts.tile([128, 256], bf16)
        nc.sync.dma_start(out=wca, in_=wca_p[:, :])
        wcb = consts.tile([64, 64], bf16)
        nc.sync.dma_start(out=wcb, in_=wcb_p[:, :])
        idf = consts.tile([128, 128], f32)
        nc.sync.dma_start(out=idf, in_=idf_p[:, :])
        idb = consts.tile([128, 128], bf16)
        nc.sync.dma_start(out=idb, in_=idb_p[:, :])

        # ---------- phase A: x slab load + XD2 build ----------
        xsb = slab.tile([64, NR, WC], bf16, name="xsb")
        nc.vector.memset(xsb[:, :, 0:1], 0.0)
        nc.vector.memset(xsb[:, :, 129:132], 0.0)
        nc.sync.dma_start(
            out=xsb[:, :, 1:129],
            in_=xg_p[:, :].rearrange("c (r w) -> c r w", r=NR))

        a_ctx = ExitStack()
        apool = a_ctx.enter_context(tc.tile_pool(name="apool", bufs=1))

        # border zero-fill: only slots the gather can read but no row write
        # covers: col0 subs(0,0)/(1,0), col128 subs(0,1)/(1,1), col129 all.
        zb = apool.tile([74, 256], bf16, name="zb")
        nc.vector.memset(zb, 0.0)
        nc.sync.dma_start(
            out=bass.AP(tensor=xd2, offset=0 * 256,
                        ap=[[WC * 256, NR], [128, 2], [1, 64]]),
            in_=zb[:, 0:128].rearrange("r (a b) -> r a b", a=2))
        nc.sync.dma_start(
            out=bass.AP(tensor=xd2, offset=128 * 256 + 64,
                        ap=[[WC * 256, NR], [128, 2], [1, 64]]),
            in_=zb[:, 0:128].rearrange("r (a b) -> r a b", a=2))
        nc.sync.dma_start(
            out=bass.AP(tensor=xd2, offset=129 * 256,
                        ap=[[WC * 256, NR], [1, 256]]),
            in_=zb)

        # host-transposed HWC slab -> write the 4 sub-blocks of XD2
        xhw = slab.tile([128, NR, 64], bf16, name="xhw")
        nc.sync.dma_start(
            out=xhw, in_=xhw_p[:, :].rearrange("w (r c) -> w r c", r=NR))
        # sub (0,0): slot (k, c=w+1) bytes [0,64), rows 0..73
        nc.sync.dma_start(
            out=bass.AP(tensor=xd2, offset=(0 * WC + 1) * 256 + 0,
                        ap=[[256, 128], [WC * 256, NR], [1, 64]]),
            in_=xhw)
        # sub (0,1): slot (k, c=w) bytes [64,128)
        nc.sync.dma_start(
            out=bass.AP(tensor=xd2, offset=(0 * WC + 0) * 256 + 64,
                        ap=[[256, 128], [WC * 256, NR], [1, 64]]),
            in_=xhw)
        # sub (1,0): slot (k-1, c=w+1) bytes [128,192), slot rows 0..72
        nc.sync.dma_start(
            out=bass.AP(tensor=xd2, offset=(0 * WC + 1) * 256 + 128,
                        ap=[[256, 128], [WC * 256, NR - 1], [1, 64]]),
            in_=xhw[:, 1:NR, :])
        # sub (1,1): slot (k-1, c=w) bytes [192,256)
        nc.sync.dma_start(
            out=bass.AP(tensor=xd2, offset=(0 * WC + 0) * 256 + 192,
                        ap=[[256, 128], [WC * 256, NR - 1], [1, 64]]),
            in_=xhw[:, 1:NR, :])
        a_ctx.close()

        # ---------- pools for phases B/C/D ----------
        ps_c = ctx.enter_context(tc.tile_pool(name="ps_c", bufs=2, space="PSUM"))
        ps_t = ctx.enter_context(tc.tile_pool(name="ps_t", bufs=2, space="PSUM"))
        ps_x = ctx.enter_context(tc.tile_pool(name="ps_x", bufs=2, space="PSUM"))
        ps_o = ctx.enter_context(tc.tile_pool(name="ps_o", bufs=2, space="PSUM"))
        bpool = ctx.enter_context(tc.tile_pool(name="bpool", bufs=2))
        cpool = ctx.enter_context(tc.tile_pool(name="cpool", bufs=2))
        tmpp = ctx.enter_context(tc.tile_pool(name="tmpp", bufs=4))
        ipool = ctx.enter_context(tc.tile_pool(name="ipool", bufs=2))
        gpool = ctx.enter_context(tc.tile_pool(name="gpool", bufs=6))
        dpool = ctx.enter_context(tc.tile_pool(name="dpool", bufs=2))
        xopool = ctx.enter_context(tc.tile_pool(name="xopool", bufs=2))
        rpool = ctx.enter_context(tc.tile_pool(name="rpool", bufs=2))
        opool = ctx.enter_context(tc.tile_pool(name="opool", bufs=2))
        jpool = ctx.enter_context(tc.tile_pool(name="jpool", bufs=4))

        # pre-drain XD2-ready onto Pool (gather ISA supports few sem waits)
        j1 = jpool.tile([16, 8], bf16, tag="j1", name="j1")
        nc.sync.dma_start(out=j1[0:1, 0:8], in_=xd2[0:1, 0:8])
        j3 = jpool.tile([16, 8], bf16, tag="j3", name="j3")
        nc.gpsimd.tensor_copy(j3[0:1, 0:4], j1[0:1, 0:4])

        def phase_b(tcn):
            off_sb = bpool.tile([18, 16, 128], f32, tag="offsb")
            for tb in range(4):
                psc = ps_c.tile([18, 512], f32, tag="c")
                for dy in range(3):
                    for dx in range(3):
                        tap = dy * 3 + dx
                        nc.tensor.matmul(
                            psc[:, :],
                            woff[:, tap, :],
                            bass.AP(tensor=xsb.tensor,
                                    offset=xsb.offset
                                    + (tcn * 16 + tb * 4 + dy + 4) * WC + dx,
                                    ap=[xsb.ap[0], [WC, 4], [1, 128]]),
                            start=(tap == 0), stop=(tap == 8))
                nc.any.tensor_copy(
                    off_sb[:, tb * 4:tb * 4 + 4, :],
                    psc[:, :].rearrange("p (r w) -> p r w", r=4))
            pst = ps_t.tile([128, 288], f32, tag="t")
            for j in range(16):
                nc.tensor.transpose(
                    pst[:, j * 18:(j + 1) * 18], off_sb[:, j, :],
                    idf[0:18, 0:18])
            offt = cpool.tile([128, 16, 18], f32, tag="offt")
            nc.any.tensor_copy(
                offt, pst[:, :].rearrange("p (t c) -> p t c", t=16))
            return offt

        def phase_c(tcn, offt):
            P = cpool.tile([128, 16, 18], f32, tag="P")
            nc.vector.tensor_tensor(
                P, offt, base2[:, tcn * 16:(tcn + 1) * 16, :], A.add)
            q_i = tmpp.tile([128, 16, 18], i32, tag="ct", name="qi")
            nc.vector.tensor_copy(q_i, P)
            Qf0 = tmpp.tile([128, 16, 18], f32, tag="ct", name="qf0")
            nc.vector.tensor_copy(Qf0, q_i)
            GT = tmpp.tile([128, 16, 18], f32, tag="ct", name="gt")
            nc.vector.tensor_tensor(GT, Qf0, P, A.is_gt)
            Qf = cpool.tile([128, 16, 18], f32, tag="Qf")
            nc.vector.tensor_tensor(Qf, Qf0, GT, A.subtract)
            FR = tmpp.tile([128, 16, 18], f32, tag="ct", name="fr")
            nc.vector.tensor_tensor(FR, P, Qf, A.subtract)
            INR = tmpp.tile([128, 16, 18], f32, tag="ct", name="inr")
            nc.vector.tensor_scalar(INR[:, :, 0:9], P[:, :, 0:9],
                                    xsc[:, 0:1], None, A.is_ge)
            nc.vector.tensor_scalar(INR[:, :, 9:18], P[:, :, 9:18],
                                    9.0, None, A.is_ge)
            INH = tmpp.tile([128, 16, 18], f32, tag="ct", name="inh")
            nc.vector.tensor_scalar(INH[:, :, 0:9], P[:, :, 0:9],
                                    xsc[:, 1:2], None, A.is_le)
            nc.vector.tensor_scalar(INH[:, :, 9:18], P[:, :, 9:18],
                                    136.0, None, A.is_le)
            nc.vector.tensor_tensor(INR, INR, INH, A.mult)
            FRV = cpool.tile([128, 16, 18], f32, tag="FRV")
            nc.vector.tensor_tensor(FRV, FR, INR, A.mult)
            QC = cpool.tile([128, 16, 18], f32, tag="QC")
            nc.vector.tensor_scalar(QC[:, :, 0:9], Qf[:, :, 0:9],
                                    4.0, 76.0, A.max, A.min)
            nc.vector.tensor_scalar(QC[:, :, 9:18], Qf[:, :, 9:18],
                                    8.0, 137.0, A.max, A.min)
            LINF = cpool.tile([128, 16, 9], f32, tag="LINF")
            nc.vector.tensor_scalar(LINF, QC[:, :, 0:9], 132.0, -536.0,
                                    A.mult, A.add)
            nc.vector.tensor_tensor(LINF, LINF, QC[:, :, 9:18], A.add)
            # n-major int16 idx [128 w, 9 n, 16 t]
            gpre = ipool.tile([128, 9, 16], i16, tag="gpre")
            nc.vector.tensor_copy(
                gpre,
                bass.AP(tensor=LINF.tensor, offset=LINF.offset,
                        ap=[LINF.ap[0], [1, 9], [9, 16]]))
            # relayout via DRAM: gstage[pl, (ph, n, tt)]; the load DMA reads
            # it back in (n, tt, ph) order so sg2 is gather-ready.
            gst_off = tcn * 1152
            for ph in range(8):
                sl = gpre[ph * 16:ph * 16 + 16]
                nc.sync.dma_start(
                    out=bass.AP(tensor=gstage, offset=gst_off + ph * 144,
                                ap=[[4608, 16], [1, 144]]),
                    in_=bass.AP(tensor=sl.tensor, offset=sl.offset,
                                ap=[sl.ap[0], [1, 144]]))
            sg = ipool.tile([128, 8, 144], i16, tag="sg")
            for grp in range(8):
                nc.sync.dma_start(
                    out=sg[grp * 16:(grp + 1) * 16, :, :],
                    in_=bass.AP(tensor=gstage, offset=gst_off,
                                ap=[[4608, 16], [144, 8], [1, 144]]))
            # ph-interleave on GPSIMD: sg2[p, n, tt*8+ph] = sg[p, ph, n*16+tt]
            sg2 = ipool.tile([128, 9, 128], i16, tag="sg2")
            nc.gpsimd.tensor_copy(
                bass.AP(tensor=sg2.tensor, offset=sg2.offset,
                        ap=[sg2.ap[0], [128, 9], [8, 16], [1, 8]]),
                bass.AP(tensor=sg.tensor, offset=sg.offset,
                        ap=[sg.ap[0], [16, 9], [1, 16], [144, 8]]))
            return FRV, sg2

        def issue_gathers(tcn, sg2):
            gs = []
            for n in range(9):
                g = gpool.tile([128, 16, 2, 2, 64], bf16, tag="g")
                nc.gpsimd.dma_gather(
                    out_ap=g.rearrange("p a b c d -> p a (b c d)"),
                    in_ap=xd2[:, :],
                    idxs_ap=sg2[:, n, :],
                    num_idxs=2048,
                    num_idxs_reg=2048,
                    elem_size=256,
                    single_packet=False,
                    queue_num=(n + tcn) % 4,
                )
                gs.append(g)
            return gs

        def combine_n(st, n):
            g = st["gs"][n]
            FRV = st["FRV"]
            xoff = st["xoff"]
            D = dpool.tile([128, 16, 2, 64], bf16, tag="D")
            nc.vector.tensor_tensor(
                D, g[:, :, 1, :, :], g[:, :, 0, :, :], A.subtract)
            H = dpool.tile([128, 16, 2, 64], bf16, tag="H")
            for tt in range(12):
                nc.vector.scalar_tensor_tensor(
                    H[:, tt, :, :], D[:, tt, :, :],
                    FRV[:, tt, n:n + 1], g[:, tt, 0, :, :],
                    A.mult, A.add)
            # last 4 rows: ACT does the scale, DVE one batched add
            for tt in range(12, 16):
                nc.scalar.mul(D[:, tt, :, :], D[:, tt, :, :],
                              FRV[:, tt, n:n + 1])
            nc.vector.tensor_tensor(
                H[:, 12:16, :, :], D[:, 12:16, :, :],
                g[:, 12:16, 0, :, :], A.add)
            D2 = dpool.tile([128, 16, 64], bf16, tag="D2")
            nc.vector.tensor_tensor(
                D2, H[:, :, 1, :], H[:, :, 0, :], A.subtract)
            for tt in range(16):
                nc.scalar.mul(D2[:, tt, :], D2[:, tt, :],
                              FRV[:, tt, 9 + n:10 + n])
            nc.vector.tensor_tensor(
                xoff[:, :, n, :], H[:, :, 0, :], D2, A.add)

        def conv_out(st):
            tcn = st["tcn"]
            xoff = st["xoff"]
            outb = opool.tile([64, 4, 512], f32, tag="outb")
            for g4 in range(4):
                rhs = rpool.tile([128, 5, 512], bf16, tag="rhs")
                for jc in range(4):
                    psx = ps_x.tile([128, 4, 128], bf16, tag="x")
                    for ti in range(4):
                        tt = g4 * 4 + ti
                        nc.tensor.transpose(
                            psx[:, ti, :],
                            xoff[:, tt, 2 * jc:2 * jc + 2, :].rearrange(
                                "p a b -> p (a b)"),
                            idb)
                    nc.any.tensor_copy(
                        rhs[:, jc, :].rearrange("p (a b) -> p a b", a=4), psx)
                psx9 = ps_x.tile([128, 4, 128], bf16, tag="x")
                for ti in range(4):
                    tt = g4 * 4 + ti
                    nc.tensor.transpose(
                        psx9[0:64, ti, :], xoff[:, tt, 8, :], idb)
                nc.any.tensor_copy(
                    rhs[0:64, 4, :].rearrange("p (a b) -> p a b", a=4),
                    psx9[0:64, :, :])
                pso = ps_o.tile([64, 512], f32, tag="o")
                for jc in range(4):
                    nc.tensor.matmul(pso, wca[:, jc * 64:(jc + 1) * 64],
                                     rhs[:, jc, :], start=(jc == 0), stop=False)
                nc.tensor.matmul(pso, wcb, rhs[0:64, 4, :],
                                 start=False, stop=True)
                nc.any.tensor_copy(outb[:, g4, :], pso)
            # ACT-queue DMA: keeps the sync queue free for idx staging
            nc.scalar.dma_start(
                out=out_p[:, tcn * 2048:(tcn + 1) * 2048],
                in_=outb.rearrange("c a b -> c (a b)"))

        prev = None
        for tcn in range(4):
            offt = phase_b(tcn)
            FRV, sg2 = phase_c(tcn, offt)
            if prev is not None:
                for n in range(3):
                    combine_n(prev, n)
            gs = issue_gathers(tcn, sg2)
            xoff = xopool.tile([128, 16, 9, 64], bf16, tag="xoff", name="xoff")
            cur = {"tcn": tcn, "gs": gs, "FRV": FRV, "xoff": xoff}
            if prev is not None:
                for n in range(3, 9):
                    combine_n(prev, n)
                conv_out(prev)
            prev = cur
        for n in range(9):
            combine_n(prev, n)
        conv_out(prev)

    nc.finalize()
    _PROGRAM = nc
    return nc


def _host_consts(W_off, b_off, W_conv):
    idxr = np.concatenate([np.arange(0, 18, 2), np.arange(1, 18, 2)])
    W_off_r = W_off[idxr]            # (18, 64, 3, 3)
    b_off_r = b_off[idxr]            # (18,)
    woff = np.ascontiguousarray(
        W_off_r.transpose(2, 3, 1, 0).reshape(9, 64, 18).transpose(1, 0, 2)
    ).reshape(64, 9 * 18).astype(BF16)
    # base2 [128 w, 64 t, 18]
    nidx = np.arange(9)
    pnx = (nidx // 3) - 1
    pny = (nidx % 3) - 1
    tt = np.arange(64)
    ww = np.arange(128)
    base2 = np.zeros((128, 64, 18), np.float32)
    base2[:, :, 0:9] = tt[None, :, None] + 9 + pnx[None, None, :] + \
        b_off_r[None, None, 0:9]
    base2[:, :, 9:18] = ww[:, None, None] + 9 + pny[None, None, :] + \
        b_off_r[None, None, 9:18]
    base2 = base2.reshape(128, 64 * 18)
    # final conv weights
    Wmat = W_conv.reshape(64, 64, 9).transpose(0, 2, 1)   # (co, n, ci)
    wca = np.zeros((128, 256), np.float32)
    for jc in range(4):
        for dn in range(2):
            wca[dn * 64:(dn + 1) * 64, jc * 64:(jc + 1) * 64] = \
                Wmat[:, 2 * jc + dn, :].T
    wcb = np.ascontiguousarray(Wmat[:, 8, :].T)           # (ci, co)
    return {
        "woff": woff,
        "base2": base2,
        "wconv_a": wca.astype(BF16),
        "wconv_b": wcb.astype(BF16),
        "ident_f": np.eye(128, dtype=np.float32),
        "ident_b": np.eye(128, dtype=np.float32).astype(BF16),
    }


def _per_core_inputs(x, consts, s, half):
    h0 = 64 * half
    xs = x[s]                                    # (64, 128, 128)
    xgs = np.zeros((64, NR, 128), BF16)
    lo = h0 - 5                                  # unpadded row of slab row 0
    r0 = max(0, lo)
    r1 = min(128, lo + NR)
    xgs[:, r0 - lo:r1 - lo, :] = xs[:, r0:r1, :].astype(BF16)
    xhw = np.ascontiguousarray(xgs.transpose(2, 1, 0))   # (128 w, 74 r, 64 c)
    xsc = np.zeros((128, 2), np.float32)
    xsc[:, 0] = 9 - h0                           # x mask lo
    xsc[:, 1] = 136 - h0                         # x mask hi
    return {
        "xg": xgs.reshape(64, NR * 128),
        "xhw": xhw.reshape(128, NR * 64),
        "xsc": xsc,
        **consts,
    }


def kernel(x, W_off, b_off, W_conv):
    _install_ntff_hook()
    # the bass kernel must run on the axon trn2 backend; undo any cpu pin
    import os
    if os.environ.get("JAX_PLATFORMS", "") == "cpu":
        try:
            import jax
            jax.config.update("jax_platforms", None)
            os.environ.pop("JAX_PLATFORMS", None)
        except Exception:
            pass
    x = np.asarray(x, np.float32)
    W_off = np.asarray(W_off, np.float32)
    b_off = np.asarray(b_off, np.float32)
    W_conv = np.asarray(W_conv, np.float32)

    from concourse.bass_utils import run_bass_kernel_spmd
    nc = _build_program()
    consts = _host_consts(W_off, b_off, W_conv)
    in_maps = [
        _per_core_inputs(x, consts, core // 2, core % 2) for core in range(NCORES)
    ]
    res = run_bass_kernel_spmd(nc, in_maps, list(range(NCORES)))
    out = np.empty((4, 64, 128, 128), np.float32)
    for core in range(NCORES):
        s, half = core // 2, core % 2
        out[s, :, 64 * half:64 * half + 64, :] = \
            res.results[core]["out"].reshape(64, 64, 128)
    return out
